# revision 52
# baseline (speedup 1.0000x reference)
"""GAT attention layer (nn_AttentionLayer) on 8 Trainium2 NeuronCores.

Row-sharded outputs: core c owns output rows I_c = [c*N/8, (c+1)*N/8).
Inputs are laid out transposed on the host (same values, column-major
shards — a sharding/layout choice): each core receives
    adjT  = adj[I_c, :].T          [N, N/8]   int32
    featT = features.T             [D, N]     f32   (replicated)
    featT_loc = features[I_c].T    [D, N/8]   f32
so the device needs NO transposes, NO collectives — one pure stream.

hs = [s2|h] per 128-row j-tile (PE fp16; featT chunks interleaved with
adj quads on one SWDGE ring, hs emitted just-in-time per chunk).
Per 512-row j-quad (j on partitions, local i on the free axis):
    z  = s1_i + s2_j              (DVE tensor_scalar per tile, s2 scalar)
    y  = leaky_relu(z)            (split: ACT Prelu(alpha) / DVE mul+max)
    e  = exp(y - 4)               (ACT, const bias)
    P  = adj * e                  (DVE; masked lanes exact 0)
    psoT[65,1024] += [h|ones]-stationary @ P^T-moving  (PE fp16, 512-wide)
Epilogue: PE-transpose psoT back to [i, 65]; out = elu(num * rcp(den)).
"""

import os
import sys

for _p in ("/opt/trn_rl_repo",):
    if os.path.isdir(_p) and _p not in sys.path:
        sys.path.append(_p)

import numpy as np

import concourse.bass as bass
import concourse.bacc as bacc
import concourse.mybir as mybir
import concourse.tile as tile
import concourse.masks as masks
from concourse import bass_utils

N, D, F = 8192, 256, 64
NCORES = 8
RL = N // NCORES
CSHIFT = 4.0    # exp range shift
ALPHA = 0.2     # leaky_relu slope
SIM_SAFE = False  # True: all-DVE leaky (CoreSim lacks Prelu); False: split w/ ACT

f32 = mybir.dt.float32
fp16 = mybir.dt.float16
fp8 = mybir.dt.float8e4
i32 = mybir.dt.int32
Alu = mybir.AluOpType
Act = mybir.ActivationFunctionType

LAST_RESULTS = None
_CACHE = {}


def _kernel_body(tc, out_d, featT_d, featTl_d, adjT_d, W_d, a_d, n=N, rl=RL):
    nc = tc.nc
    nit = rl // 128           # local i-tiles
    njt = n // 128            # global j-tiles
    nk = D // 128             # d contraction tiles
    QT = 4                    # j-tiles per chain quad
    NQ = njt // QT
    HW = F + 1                # hs8 cols: h(64) | ones
    NXC = 4                   # X^T streamed in chunks along j
    jxc = n // NXC
    tpc = njt // NXC          # j-tiles per featT chunk
    AQB = min(14, NQ)         # adj quad ring buffers (deep enough that
                              # dispatch buffer-waits resolve before the ring
                              # reaches them, so it never starves)

    with (
        tc.tile_pool(name="sbP", bufs=1) as sbP,
        tc.tile_pool(name="sbA", bufs=AQB) as sbA,
        tc.tile_pool(name="sbU", bufs=3) as sbU,
        tc.tile_pool(name="sbE", bufs=4) as sbE,
        tc.tile_pool(name="pp", bufs=1, space="PSUM") as pp,
    ):
        # ---- SWDGE ring: chunk 0 first (longest dependent chain), local X^T,
        # then featT chunks just-in-time between adj quads
        xTl = sbP.tile([128, nk, rl], fp16)
        xTf = [
            sbP.tile([128, nk, jxc], fp16, name=f"xTf{c}") for c in range(NXC)
        ]
        ftr = featT_d.rearrange("(k p) (c j) -> c p k j", p=128, c=NXC)
        aq = [
            sbA.tile([128, QT, rl], fp16, tag="aq", name=f"aq{q}") for q in range(NQ)
        ]
        aqr = adjT_d.rearrange("(Q t p) i -> Q p t i", t=QT, p=128)
        nc.gpsimd.dma_start(xTf[0][:], ftr[0])
        nc.gpsimd.dma_start(xTl[:], featTl_d.rearrange("(k p) i -> p k i", p=128))

        # identities for the PE transposes; after the first two dispatches so
        # they don't delay the ring start (their consumers have slack)
        ident = sbP.tile([HW, HW], f32)
        masks.make_identity(nc, ident[:])
        ident128 = sbP.tile([128, 128], fp16)
        masks.make_identity(nc, ident128[:])

        qi = 0
        for c in range(NXC):
            if c:
                nc.gpsimd.dma_start(xTf[c][:], ftr[c])
            for _ in range(2):
                if qi < NQ:
                    nc.gpsimd.dma_start(aq[qi][:], aqr[qi])
                    qi += 1
        while qi < NQ:
            nc.gpsimd.dma_start(aq[qi][:], aqr[qi])
            qi += 1

        # ---- constants ----------------------------------------------------
        cshift = sbP.tile([128, 1], f32)
        nc.vector.memset(cshift[:], -CSHIFT)
        arow = sbP.tile([1, 2 * F], f32)
        nc.sync.dma_start(arow[:], a_d.rearrange("f o -> o f"))
        onesf = sbP.tile([1, 128], f32)
        nc.vector.memset(onesf[:], 1.0)
        ab = sbP.tile([128, 2 * F], f32)
        psab = pp.tile([128, 2 * F], f32, tag="pro", name="psab", bufs=2)
        nc.tensor.matmul(psab[:], onesf[:], arow[:])
        nc.vector.tensor_copy(ab[:], psab[:])
        wsb = sbP.tile([128, nk, F], f32)
        nc.sync.dma_start(wsb[:], W_d.rearrange("(k p) f -> p k f", p=128))
        wa = sbP.tile([128, nk, 2], f32)
        scr = sbP.tile([128, F], f32)
        for k in range(nk):
            # rhs16 col F = W@a2 (s2 of all rows), col F+1 = W@a1 (s1 local)
            nc.vector.scalar_tensor_tensor(
                scr[:], wsb[:, k, :], 1.0, ab[:, F:], Alu.mult, Alu.mult,
                accum_out=wa[:, k, 0:1],
            )
            nc.vector.scalar_tensor_tensor(
                scr[:], wsb[:, k, :], 1.0, ab[:, :F], Alu.mult, Alu.mult,
                accum_out=wa[:, k, 1:2],
            )
        # rhs16 cols: [W@a2 | W | W@a1] so psh comes out [s2 | h]
        rhs16 = sbP.tile([128, nk, F + 2], fp16)
        for k in range(nk):
            nc.vector.tensor_copy(rhs16[:, k, 0:1], wa[:, k, 0:1])
            nc.vector.tensor_copy(rhs16[:, k, 1 : F + 1], wsb[:, k, :])
            nc.vector.tensor_copy(rhs16[:, k, F + 1 : F + 2], wa[:, k, 1:2])

        # ---- s1 local -> DRAM bounce -> free-axis broadcast tile ----------
        s1c16 = sbP.tile([128, nit], fp16)
        for it in range(nit):
            ps1 = pp.tile([128, 1], f32, tag="pro", name=f"ps1_{it}", bufs=2)
            for k in range(nk):
                nc.tensor.matmul(
                    ps1[:], xTl[:, k, it * 128 : (it + 1) * 128],
                    rhs16[:, k, F + 1 : F + 2],
                    start=(k == 0), stop=(k == nk - 1),
                )
            nc.vector.tensor_copy(s1c16[:, it : it + 1], ps1[:])
        # s1 column -> row without a DRAM bounce: PE transpose, then one
        # tiny SBUF->SBUF DMA to flatten the nit partitions into one row
        pst = pp.tile([nit, 128], fp16, tag="pro", name="pst", bufs=2)
        nc.tensor.transpose(pst[:], s1c16[:], ident128[:])
        psrowSB = sbP.tile([nit, 128], fp16)
        nc.vector.tensor_copy(psrowSB[:], pst[:])
        s1row = sbP.tile([1, rl], fp16)
        nc.sync.dma_start(
            s1row[:].rearrange("o (t i) -> o t i", t=nit), psrowSB[:]
        )
        ones1 = sbP.tile([1, 128], fp16)
        nc.vector.memset(ones1[:], 1.0)
        s1b = sbP.tile([128, rl], fp16)
        for cc0 in range(0, rl, 512):
            wch = min(512, rl - cc0)
            psb = pp.tile([128, wch], f32, tag="pro", name=f"psb{cc0}", bufs=2)
            nc.tensor.matmul(psb[:], ones1[:], s1row[:, cc0 : cc0 + wch])
            nc.vector.tensor_copy(s1b[:, cc0 : cc0 + wch], psb[:])

        # ---- hs16 [s2|h|ones]; stationary slice is cols 1: = [h|ones] -----
        hs8 = sbP.tile([128, njt, F + 2], fp16)
        nc.vector.memset(hs8[:, :, F + 1 : F + 2], 1.0)
        s2c = sbP.tile([128, njt], f32)

        w_lo = min(512, rl)
        pso_lo = pp.tile([HW, w_lo], f32, tag="lo", name="pso_lo")
        pso_hi = pp.tile([HW, rl - 512], f32, tag="hi", name="pso_hi") \
            if rl > 512 else None

        # hs emitted in half-chunk units, ~2 quads ahead of first use, so a
        # quad never queues behind more than half a chunk of copies/casts
        hpc = max(1, tpc // 2)    # j-tiles per emission unit
        nun = njt // hpc
        done_units = 0

        for q in range(NQ):
            need = min(nun, ((q + 2) * QT + hpc - 1) // hpc)
            while done_units < need:
                u = done_units
                c = (u * hpc) // tpc
                for tth in range(hpc):
                    t = u * hpc + tth
                    psh = pp.tile([128, F + 1], f32, tag="psh", name=f"psh{t}",
                                  bufs=2)
                    for k in range(nk):
                        nc.tensor.matmul(
                            psh[:],
                            xTf[c][:, k, (t - c * tpc) * 128
                                   : (t - c * tpc + 1) * 128],
                            rhs16[:, k, 0 : F + 1],
                            start=(k == 0), stop=(k == nk - 1),
                        )
                    nc.scalar.copy(hs8[:, t, 0 : F + 1], psh[:])
                sl = slice(u * hpc, (u + 1) * hpc)
                nc.vector.tensor_copy(s2c[:, sl], hs8[:, sl, 0])
                done_units += 1

            w = aq[q]  # in-place: adj tile becomes the masked-P tile
            u = sbU.tile([128, QT, rl], fp16, tag="u", name=f"u{q}")
            for tt in range(QT):
                t = q * QT + tt
                nc.vector.tensor_scalar(
                    u[:, tt, :], s1b[:], s2c[:, t : t + 1], None, Alu.add
                )
            # leaky_relu: front span on ACT (exact Prelu), rest native DVE
            wf = w[:].rearrange("p t i -> p (t i)")
            uf = u[:].rearrange("p t i -> p (t i)")
            sp_a = 0 if SIM_SAFE else (QT * rl) // 2
            if sp_a:
                nc.scalar.activation(
                    uf[:, 0:sp_a], uf[:, 0:sp_a], Act.Prelu,
                    bias=0.0, scale=1.0, alpha=ALPHA,
                )
            ud = uf[:, sp_a : QT * rl]
            lt = sbU.tile([128, QT * rl - sp_a], fp16, tag="lt", name=f"lt{q}",
                          bufs=2)
            nc.vector.tensor_scalar_mul(lt[:], ud, ALPHA)
            nc.vector.tensor_tensor(ud, ud, lt[:], Alu.max)
            nc.scalar.activation(uf, uf, Act.Exp, bias=cshift[:], scale=1.0)
            nc.vector.tensor_tensor(wf, wf, uf, Alu.mult)
            for tt in range(QT):
                t = q * QT + tt
                st, sp = (t == 0), (t == njt - 1)
                nc.tensor.matmul(
                    pso_lo[:], hs8[:, t, 1 : F + 2], w[:, tt, 0:w_lo],
                    start=st, stop=sp,
                )
                if pso_hi is not None:
                    nc.tensor.matmul(
                        pso_hi[:], hs8[:, t, 1 : F + 2], w[:, tt, 512:rl],
                        start=st, stop=sp,
                    )

        # ---- epilogue: transpose psoT back, divide, elu -------------------
        psoSB = sbP.tile([HW, rl], f32)
        nc.vector.tensor_copy(psoSB[:, 0:w_lo], pso_lo[:])
        if pso_hi is not None:
            nc.vector.tensor_copy(psoSB[:, 512:rl], pso_hi[:])
        # per-chunk: transpose + divide; elu batched across all chunks so the
        # tail is 4 wide ops + one DMA instead of 32 tiny ops + 8 DMAs
        stage = sbE.tile([128, nit, F], f32, tag="stage", bufs=1)
        for it in range(nit):
            psT = pp.tile([128, HW], f32, tag="psT", name=f"psT{it}", bufs=2)
            nc.tensor.transpose(
                psT[:], psoSB[:, it * 128 : (it + 1) * 128], ident[:]
            )
            rcp = sbE.tile([128, 1], f32, tag="rcp")
            nc.vector.reciprocal(rcp[:], psT[:, F : F + 1])
            nc.vector.tensor_scalar_mul(stage[:, it, :], psT[:, 0:F], rcp[:])
        sf = stage[:].rearrange("p t f -> p (t f)")
        rt = sbE.tile([128, nit * F], f32, tag="rt", bufs=1)
        nc.vector.tensor_scalar_max(rt[:], sf, 0.0)
        nc.vector.tensor_scalar_min(sf, sf, 0.0)
        nc.scalar.activation(sf, sf, Act.Exp)
        nc.vector.scalar_tensor_tensor(rt[:], sf, -1.0, rt[:], Alu.add, Alu.add)
        nc.sync.dma_start(
            out_d.rearrange("(t p) f -> p t f", p=128),
            rt[:].rearrange("p (t f) -> p t f", t=nit),
        )


def _build(n=N, rl=RL, ncores=NCORES):
    key = (n, rl, ncores)
    if key in _CACHE:
        return _CACHE[key]
    nc = bacc.Bacc(
        "TRN2", target_bir_lowering=False, debug=False, num_devices=ncores
    )
    featT = nc.dram_tensor("featT", [D, n], f32, kind="ExternalInput").ap()
    featTl = nc.dram_tensor("featTl", [D, rl], f32, kind="ExternalInput").ap()
    adjT = nc.dram_tensor("adjT", [n, rl], i32, kind="ExternalInput").ap()
    W = nc.dram_tensor("W", [D, F], f32, kind="ExternalInput").ap()
    a = nc.dram_tensor("a", [2 * F, 1], f32, kind="ExternalInput").ap()
    out = nc.dram_tensor("out", [rl, F], f32, kind="ExternalOutput").ap()
    with tile.TileContext(nc) as tc:
        _kernel_body(tc, out, featT, featTl, adjT, W, a, n=n, rl=rl)
    nc.compile()
    _CACHE[key] = nc
    return nc


def kernel(features, adj, W, a):
    global LAST_RESULTS
    features = np.ascontiguousarray(features, dtype=np.float32)
    adj = np.ascontiguousarray(adj, dtype=np.int32)
    W = np.ascontiguousarray(W, dtype=np.float32)
    a = np.ascontiguousarray(a, dtype=np.float32)

    n = adj.shape[0]
    rl = n // NCORES
    nc = _build(n=n, rl=rl, ncores=NCORES)
    featT = np.ascontiguousarray(features.T)
    in_maps = [
        {
            "featT": featT,
            "featTl": np.ascontiguousarray(features[c * rl : (c + 1) * rl].T),
            "adjT": np.ascontiguousarray(adj[c * rl : (c + 1) * rl].T),
            "W": W,
            "a": a,
        }
        for c in range(NCORES)
    ]
    res = bass_utils.run_bass_kernel_spmd(nc, in_maps, core_ids=list(range(NCORES)))
    LAST_RESULTS = res
    return np.concatenate([res.results[c]["out"] for c in range(NCORES)], axis=0)


# revision 53
# speedup vs baseline: 1.0169x; 1.0169x over previous
"""GAT attention layer (nn_AttentionLayer) on 8 Trainium2 NeuronCores.

Row-sharded outputs: core c owns output rows I_c = [c*N/8, (c+1)*N/8).
Inputs are laid out transposed on the host (same values, column-major
shards — a sharding/layout choice): each core receives
    adjT  = adj[I_c, :].T          [N, N/8]   int32
    featT = features.T             [D, N]     f32   (replicated)
    featT_loc = features[I_c].T    [D, N/8]   f32
so the device needs NO transposes, NO collectives — one pure stream.

hs = [s2|h] per 128-row j-tile (PE fp16; featT chunks interleaved with
adj quads on one SWDGE ring, hs emitted just-in-time per chunk).
Per 512-row j-quad (j on partitions, local i on the free axis):
    z  = s1_i + s2_j              (DVE tensor_scalar per tile, s2 scalar)
    y  = leaky_relu(z)            (split: ACT Prelu(alpha) / DVE mul+max)
    e  = exp(y - 4)               (ACT, const bias)
    P  = adj * e                  (DVE; masked lanes exact 0)
    psoT[65,1024] += [h|ones]-stationary @ P^T-moving  (PE fp16, 512-wide)
Epilogue: PE-transpose psoT back to [i, 65]; out = elu(num * rcp(den)).
"""

import os
import sys

for _p in ("/opt/trn_rl_repo",):
    if os.path.isdir(_p) and _p not in sys.path:
        sys.path.append(_p)

import numpy as np

import concourse.bass as bass
import concourse.bacc as bacc
import concourse.mybir as mybir
import concourse.tile as tile
import concourse.masks as masks
from concourse import bass_utils

N, D, F = 8192, 256, 64
NCORES = 8
RL = N // NCORES
CSHIFT = 4.0    # exp range shift
ALPHA = 0.2     # leaky_relu slope
SIM_SAFE = False  # True: all-DVE leaky (CoreSim lacks Prelu); False: split w/ ACT

f32 = mybir.dt.float32
fp16 = mybir.dt.float16
fp8 = mybir.dt.float8e4
i32 = mybir.dt.int32
Alu = mybir.AluOpType
Act = mybir.ActivationFunctionType

LAST_RESULTS = None
_CACHE = {}


def _kernel_body(tc, out_d, featT_d, featTl_d, adjT_d, W_d, a_d, n=N, rl=RL):
    nc = tc.nc
    nit = rl // 128           # local i-tiles
    njt = n // 128            # global j-tiles
    nk = D // 128             # d contraction tiles
    QT = 4                    # j-tiles per chain quad
    NQ = njt // QT
    HW = F + 1                # hs8 cols: h(64) | ones
    NXC = 4                   # X^T streamed in chunks along j
    jxc = n // NXC
    tpc = njt // NXC          # j-tiles per featT chunk
    AQB = min(14, NQ)         # adj quad ring buffers (deep enough that
                              # dispatch buffer-waits resolve before the ring
                              # reaches them, so it never starves)

    with (
        tc.tile_pool(name="sbP", bufs=1) as sbP,
        tc.tile_pool(name="sbA", bufs=AQB) as sbA,
        tc.tile_pool(name="sbU", bufs=3) as sbU,
        tc.tile_pool(name="sbE", bufs=4) as sbE,
        tc.tile_pool(name="pp", bufs=1, space="PSUM") as pp,
    ):
        # ---- SWDGE ring: chunk 0 first (longest dependent chain), local X^T,
        # then featT chunks just-in-time between adj quads
        xTl = sbP.tile([128, nk, rl], fp16)
        xTf = [
            sbP.tile([128, nk, jxc], fp16, name=f"xTf{c}") for c in range(NXC)
        ]
        ftr = featT_d.rearrange("(k p) (c j) -> c p k j", p=128, c=NXC)
        aq = [
            sbA.tile([128, QT, rl], fp16, tag="aq", name=f"aq{q}") for q in range(NQ)
        ]
        aqr = adjT_d.rearrange("(Q t p) i -> Q p t i", t=QT, p=128)
        nc.gpsimd.dma_start(xTf[0][:], ftr[0])
        nc.gpsimd.dma_start(xTl[:], featTl_d.rearrange("(k p) i -> p k i", p=128))

        # identities for the PE transposes; after the first two dispatches so
        # they don't delay the ring start (their consumers have slack)
        ident = sbP.tile([HW, HW], f32)
        masks.make_identity(nc, ident[:])
        ident128 = sbP.tile([128, 128], fp16)
        masks.make_identity(nc, ident128[:])

        qi = 0
        for c in range(NXC):
            if c:
                nc.gpsimd.dma_start(xTf[c][:], ftr[c])
            for _ in range(2):
                if qi < NQ:
                    nc.gpsimd.dma_start(aq[qi][:], aqr[qi])
                    qi += 1
        while qi < NQ:
            nc.gpsimd.dma_start(aq[qi][:], aqr[qi])
            qi += 1

        # ---- constants ----------------------------------------------------
        cshift = sbP.tile([128, 1], f32)
        nc.vector.memset(cshift[:], -CSHIFT)
        arow = sbP.tile([1, 2 * F], f32)
        nc.sync.dma_start(arow[:], a_d.rearrange("f o -> o f"))
        onesf = sbP.tile([1, 128], f32)
        nc.vector.memset(onesf[:], 1.0)
        ab = sbP.tile([128, 2 * F], f32)
        psab = pp.tile([128, 2 * F], f32, tag="pro", name="psab", bufs=2)
        nc.tensor.matmul(psab[:], onesf[:], arow[:])
        nc.vector.tensor_copy(ab[:], psab[:])
        wsb = sbP.tile([128, nk, F], f32)
        nc.sync.dma_start(wsb[:], W_d.rearrange("(k p) f -> p k f", p=128))
        wa = sbP.tile([128, nk, 2], f32)
        scr = sbP.tile([128, F], f32)
        for k in range(nk):
            # rhs16 col F = W@a2 (s2 of all rows), col F+1 = W@a1 (s1 local)
            nc.vector.scalar_tensor_tensor(
                scr[:], wsb[:, k, :], 1.0, ab[:, F:], Alu.mult, Alu.mult,
                accum_out=wa[:, k, 0:1],
            )
            nc.vector.scalar_tensor_tensor(
                scr[:], wsb[:, k, :], 1.0, ab[:, :F], Alu.mult, Alu.mult,
                accum_out=wa[:, k, 1:2],
            )
        # rhs16 cols: [W@a2 | W | W@a1] so psh comes out [s2 | h]
        rhs16 = sbP.tile([128, nk, F + 2], fp16)
        for k in range(nk):
            nc.vector.tensor_copy(rhs16[:, k, 0:1], wa[:, k, 0:1])
            nc.vector.tensor_copy(rhs16[:, k, 1 : F + 1], wsb[:, k, :])
            nc.vector.tensor_copy(rhs16[:, k, F + 1 : F + 2], wa[:, k, 1:2])

        # ---- s1 local -> DRAM bounce -> free-axis broadcast tile ----------
        s1c16 = sbP.tile([128, nit], fp16)
        for it in range(nit):
            ps1 = pp.tile([128, 1], f32, tag="pro", name=f"ps1_{it}", bufs=2)
            for k in range(nk):
                nc.tensor.matmul(
                    ps1[:], xTl[:, k, it * 128 : (it + 1) * 128],
                    rhs16[:, k, F + 1 : F + 2],
                    start=(k == 0), stop=(k == nk - 1),
                )
            nc.vector.tensor_copy(s1c16[:, it : it + 1], ps1[:])
        # s1 column -> row without a DRAM bounce: PE transpose, then one
        # tiny SBUF->SBUF DMA to flatten the nit partitions into one row
        pst = pp.tile([nit, 128], fp16, tag="pro", name="pst", bufs=2)
        nc.tensor.transpose(pst[:], s1c16[:], ident128[:])
        psrowSB = sbP.tile([nit, 128], fp16)
        nc.vector.tensor_copy(psrowSB[:], pst[:])
        s1row = sbP.tile([1, rl], fp16)
        nc.sync.dma_start(
            s1row[:].rearrange("o (t i) -> o t i", t=nit), psrowSB[:]
        )
        ones1 = sbP.tile([1, 128], fp16)
        nc.vector.memset(ones1[:], 1.0)
        s1b = sbP.tile([128, rl], fp16)
        for cc0 in range(0, rl, 512):
            wch = min(512, rl - cc0)
            psb = pp.tile([128, wch], f32, tag="pro", name=f"psb{cc0}", bufs=2)
            nc.tensor.matmul(psb[:], ones1[:], s1row[:, cc0 : cc0 + wch])
            nc.vector.tensor_copy(s1b[:, cc0 : cc0 + wch], psb[:])

        # ---- hs16 [s2|h|ones]; stationary slice is cols 1: = [h|ones] -----
        hs8 = sbP.tile([128, njt, F + 2], fp16)
        nc.vector.memset(hs8[:, :, F + 1 : F + 2], 1.0)
        s2c = sbP.tile([128, njt], f32)

        w_lo = min(512, rl)
        pso_lo = pp.tile([HW, w_lo], f32, tag="lo", name="pso_lo")
        pso_hi = pp.tile([HW, rl - 512], f32, tag="hi", name="pso_hi") \
            if rl > 512 else None

        # hs emitted in half-chunk units, ~2 quads ahead of first use, so a
        # quad never queues behind more than half a chunk of copies/casts
        hpc = max(1, tpc // 2)    # j-tiles per emission unit
        nun = njt // hpc
        done_units = 0

        for q in range(NQ):
            need = min(nun, ((q + 2) * QT + hpc - 1) // hpc)
            while done_units < need:
                u = done_units
                c = (u * hpc) // tpc
                for tth in range(hpc):
                    t = u * hpc + tth
                    psh = pp.tile([128, F + 1], f32, tag="psh", name=f"psh{t}",
                                  bufs=2)
                    for k in range(nk):
                        nc.tensor.matmul(
                            psh[:],
                            xTf[c][:, k, (t - c * tpc) * 128
                                   : (t - c * tpc + 1) * 128],
                            rhs16[:, k, 0 : F + 1],
                            start=(k == 0), stop=(k == nk - 1),
                        )
                    nc.scalar.copy(hs8[:, t, 0 : F + 1], psh[:])
                sl = slice(u * hpc, (u + 1) * hpc)
                nc.vector.tensor_copy(s2c[:, sl], hs8[:, sl, 0])
                done_units += 1

            w = aq[q]  # in-place: adj tile becomes the masked-P tile
            u = sbU.tile([128, QT, rl], fp16, tag="u", name=f"u{q}")
            for tt in range(QT):
                t = q * QT + tt
                nc.vector.tensor_scalar(
                    u[:, tt, :], s1b[:], s2c[:, t : t + 1], None, Alu.add
                )
            wf = w[:].rearrange("p t i -> p (t i)")
            uf = u[:].rearrange("p t i -> p (t i)")
            if q == NQ - 1 and not SIM_SAFE:
                # last quad: per-tile stages so the drained engines pipeline
                # the final chain instead of serializing [128,4096]-wide ops
                for tt in range(QT):
                    ut = u[:, tt, :]
                    if tt < QT // 2:
                        nc.scalar.activation(
                            ut, ut, Act.Prelu, bias=0.0, scale=1.0, alpha=ALPHA
                        )
                    else:
                        lt = sbU.tile([128, rl], fp16, tag="lt",
                                      name=f"ltz{tt}", bufs=2)
                        nc.vector.tensor_scalar_mul(lt[:], ut, ALPHA)
                        nc.vector.tensor_tensor(ut, ut, lt[:], Alu.max)
                    nc.scalar.activation(ut, ut, Act.Exp, bias=cshift[:],
                                         scale=1.0)
                    nc.vector.tensor_tensor(w[:, tt, :], w[:, tt, :], ut,
                                            Alu.mult)
            else:
                # leaky_relu: front span on ACT (exact Prelu), rest native DVE
                sp_a = 0 if SIM_SAFE else (QT * rl) // 2
                if sp_a:
                    nc.scalar.activation(
                        uf[:, 0:sp_a], uf[:, 0:sp_a], Act.Prelu,
                        bias=0.0, scale=1.0, alpha=ALPHA,
                    )
                ud = uf[:, sp_a : QT * rl]
                lt = sbU.tile([128, QT * rl - sp_a], fp16, tag="lt",
                              name=f"lt{q}", bufs=2)
                nc.vector.tensor_scalar_mul(lt[:], ud, ALPHA)
                nc.vector.tensor_tensor(ud, ud, lt[:], Alu.max)
                nc.scalar.activation(uf, uf, Act.Exp, bias=cshift[:], scale=1.0)
                nc.vector.tensor_tensor(wf, wf, uf, Alu.mult)
            for tt in range(QT):
                t = q * QT + tt
                st, sp = (t == 0), (t == njt - 1)
                nc.tensor.matmul(
                    pso_lo[:], hs8[:, t, 1 : F + 2], w[:, tt, 0:w_lo],
                    start=st, stop=sp,
                )
                if pso_hi is not None:
                    nc.tensor.matmul(
                        pso_hi[:], hs8[:, t, 1 : F + 2], w[:, tt, 512:rl],
                        start=st, stop=sp,
                    )

        # ---- epilogue: transpose psoT back, divide, elu -------------------
        psoSB = sbP.tile([HW, rl], f32)
        nc.vector.tensor_copy(psoSB[:, 0:w_lo], pso_lo[:])
        if pso_hi is not None:
            nc.vector.tensor_copy(psoSB[:, 512:rl], pso_hi[:])
        # per-chunk: transpose + divide; elu batched across all chunks so the
        # tail is 4 wide ops + one DMA instead of 32 tiny ops + 8 DMAs
        stage = sbE.tile([128, nit, F], f32, tag="stage", bufs=1)
        for it in range(nit):
            psT = pp.tile([128, HW], f32, tag="psT", name=f"psT{it}", bufs=2)
            nc.tensor.transpose(
                psT[:], psoSB[:, it * 128 : (it + 1) * 128], ident[:]
            )
            rcp = sbE.tile([128, 1], f32, tag="rcp")
            nc.vector.reciprocal(rcp[:], psT[:, F : F + 1])
            nc.vector.tensor_scalar_mul(stage[:, it, :], psT[:, 0:F], rcp[:])
        sf = stage[:].rearrange("p t f -> p (t f)")
        rt = sbE.tile([128, nit * F], f32, tag="rt", bufs=1)
        nc.vector.tensor_scalar_max(rt[:], sf, 0.0)
        nc.vector.tensor_scalar_min(sf, sf, 0.0)
        nc.scalar.activation(sf, sf, Act.Exp)
        nc.vector.scalar_tensor_tensor(rt[:], sf, -1.0, rt[:], Alu.add, Alu.add)
        nc.sync.dma_start(
            out_d.rearrange("(t p) f -> p t f", p=128),
            rt[:].rearrange("p (t f) -> p t f", t=nit),
        )


def _build(n=N, rl=RL, ncores=NCORES):
    key = (n, rl, ncores)
    if key in _CACHE:
        return _CACHE[key]
    nc = bacc.Bacc(
        "TRN2", target_bir_lowering=False, debug=False, num_devices=ncores
    )
    featT = nc.dram_tensor("featT", [D, n], f32, kind="ExternalInput").ap()
    featTl = nc.dram_tensor("featTl", [D, rl], f32, kind="ExternalInput").ap()
    adjT = nc.dram_tensor("adjT", [n, rl], i32, kind="ExternalInput").ap()
    W = nc.dram_tensor("W", [D, F], f32, kind="ExternalInput").ap()
    a = nc.dram_tensor("a", [2 * F, 1], f32, kind="ExternalInput").ap()
    out = nc.dram_tensor("out", [rl, F], f32, kind="ExternalOutput").ap()
    with tile.TileContext(nc) as tc:
        _kernel_body(tc, out, featT, featTl, adjT, W, a, n=n, rl=rl)
    nc.compile()
    _CACHE[key] = nc
    return nc


def kernel(features, adj, W, a):
    global LAST_RESULTS
    features = np.ascontiguousarray(features, dtype=np.float32)
    adj = np.ascontiguousarray(adj, dtype=np.int32)
    W = np.ascontiguousarray(W, dtype=np.float32)
    a = np.ascontiguousarray(a, dtype=np.float32)

    n = adj.shape[0]
    rl = n // NCORES
    nc = _build(n=n, rl=rl, ncores=NCORES)
    featT = np.ascontiguousarray(features.T)
    in_maps = [
        {
            "featT": featT,
            "featTl": np.ascontiguousarray(features[c * rl : (c + 1) * rl].T),
            "adjT": np.ascontiguousarray(adj[c * rl : (c + 1) * rl].T),
            "W": W,
            "a": a,
        }
        for c in range(NCORES)
    ]
    res = bass_utils.run_bass_kernel_spmd(nc, in_maps, core_ids=list(range(NCORES)))
    LAST_RESULTS = res
    return np.concatenate([res.results[c]["out"] for c in range(NCORES)], axis=0)


# revision 54
# speedup vs baseline: 1.0252x; 1.0082x over previous
"""GAT attention layer (nn_AttentionLayer) on 8 Trainium2 NeuronCores.

Row-sharded outputs: core c owns output rows I_c = [c*N/8, (c+1)*N/8).
Inputs are laid out transposed on the host (same values, column-major
shards — a sharding/layout choice): each core receives
    adjT  = adj[I_c, :].T          [N, N/8]   int32
    featT = features.T             [D, N]     f32   (replicated)
    featT_loc = features[I_c].T    [D, N/8]   f32
so the device needs NO transposes, NO collectives — one pure stream.

hs = [s2|h] per 128-row j-tile (PE fp16; featT chunks interleaved with
adj quads on one SWDGE ring, hs emitted just-in-time per chunk).
Per 512-row j-quad (j on partitions, local i on the free axis):
    z  = s1_i + s2_j              (DVE tensor_scalar per tile, s2 scalar)
    y  = leaky_relu(z)            (split: ACT Prelu(alpha) / DVE mul+max)
    e  = exp(y - 4)               (ACT, const bias)
    P  = adj * e                  (DVE; masked lanes exact 0)
    psoT[65,1024] += [h|ones]-stationary @ P^T-moving  (PE fp16, 512-wide)
Epilogue: PE-transpose psoT back to [i, 65]; out = elu(num * rcp(den)).
"""

import os
import sys

for _p in ("/opt/trn_rl_repo",):
    if os.path.isdir(_p) and _p not in sys.path:
        sys.path.append(_p)

import numpy as np

import concourse.bass as bass
import concourse.bacc as bacc
import concourse.mybir as mybir
import concourse.tile as tile
import concourse.masks as masks
from concourse import bass_utils

N, D, F = 8192, 256, 64
NCORES = 8
RL = N // NCORES
CSHIFT = 4.0    # exp range shift
ALPHA = 0.2     # leaky_relu slope
SIM_SAFE = False  # True: all-DVE leaky (CoreSim lacks Prelu); False: split w/ ACT

f32 = mybir.dt.float32
fp16 = mybir.dt.float16
fp8 = mybir.dt.float8e4
i32 = mybir.dt.int32
Alu = mybir.AluOpType
Act = mybir.ActivationFunctionType

LAST_RESULTS = None
_CACHE = {}


def _kernel_body(tc, out_d, featT_d, featTl_d, adjT_d, W_d, a_d, n=N, rl=RL):
    nc = tc.nc
    nit = rl // 128           # local i-tiles
    njt = n // 128            # global j-tiles
    nk = D // 128             # d contraction tiles
    QT = 4                    # j-tiles per chain quad
    NQ = njt // QT
    HW = F + 1                # hs8 cols: h(64) | ones
    NXC = 4                   # X^T streamed in chunks along j
    jxc = n // NXC
    tpc = njt // NXC          # j-tiles per featT chunk
    AQB = min(14, NQ)         # adj quad ring buffers (deep enough that
                              # dispatch buffer-waits resolve before the ring
                              # reaches them, so it never starves)

    with (
        tc.tile_pool(name="sbP", bufs=1) as sbP,
        tc.tile_pool(name="sbA", bufs=AQB) as sbA,
        tc.tile_pool(name="sbU", bufs=3) as sbU,
        tc.tile_pool(name="sbE", bufs=4) as sbE,
        tc.tile_pool(name="pp", bufs=1, space="PSUM") as pp,
    ):
        # ---- SWDGE ring: chunk 0 first (longest dependent chain), local X^T,
        # then featT chunks just-in-time between adj quads
        xTl = sbP.tile([128, nk, rl], fp16)
        xTf = [
            sbP.tile([128, nk, jxc], fp16, name=f"xTf{c}") for c in range(NXC)
        ]
        ftr = featT_d.rearrange("(k p) (c j) -> c p k j", p=128, c=NXC)
        aq = [
            sbA.tile([128, QT, rl], fp16, tag="aq", name=f"aq{q}") for q in range(NQ)
        ]
        aqr = adjT_d.rearrange("(Q t p) i -> Q p t i", t=QT, p=128)
        nc.gpsimd.dma_start(xTf[0][:], ftr[0])
        nc.gpsimd.dma_start(xTl[:], featTl_d.rearrange("(k p) i -> p k i", p=128))

        # identities for the PE transposes; after the first two dispatches so
        # they don't delay the ring start (their consumers have slack)
        ident = sbP.tile([HW, HW], f32)
        masks.make_identity(nc, ident[:])
        ident128 = sbP.tile([128, 128], fp16)
        masks.make_identity(nc, ident128[:])

        qi = 0
        for c in range(NXC):
            if c:
                nc.gpsimd.dma_start(xTf[c][:], ftr[c])
            for _ in range(2):
                if qi < NQ:
                    nc.gpsimd.dma_start(aq[qi][:], aqr[qi])
                    qi += 1
        while qi < NQ:
            nc.gpsimd.dma_start(aq[qi][:], aqr[qi])
            qi += 1

        # ---- constants ----------------------------------------------------
        cshift = sbP.tile([128, 1], f32)
        nc.vector.memset(cshift[:], -CSHIFT)
        # warm the ACT engine (act-table load + startup) under the DMA wait
        # so the first real activation isn't delayed by it
        nc.scalar.activation(cshift[:], cshift[:], Act.Exp)
        nc.vector.memset(cshift[:], -CSHIFT)
        arow = sbP.tile([1, 2 * F], f32)
        nc.sync.dma_start(arow[:], a_d.rearrange("f o -> o f"))
        onesf = sbP.tile([1, 128], f32)
        nc.vector.memset(onesf[:], 1.0)
        ab = sbP.tile([128, 2 * F], f32)
        psab = pp.tile([128, 2 * F], f32, tag="pro", name="psab", bufs=2)
        nc.tensor.matmul(psab[:], onesf[:], arow[:])
        nc.vector.tensor_copy(ab[:], psab[:])
        wsb = sbP.tile([128, nk, F], f32)
        nc.sync.dma_start(wsb[:], W_d.rearrange("(k p) f -> p k f", p=128))
        wa = sbP.tile([128, nk, 2], f32)
        scr = sbP.tile([128, F], f32)
        for k in range(nk):
            # rhs16 col F = W@a2 (s2 of all rows), col F+1 = W@a1 (s1 local)
            nc.vector.scalar_tensor_tensor(
                scr[:], wsb[:, k, :], 1.0, ab[:, F:], Alu.mult, Alu.mult,
                accum_out=wa[:, k, 0:1],
            )
            nc.vector.scalar_tensor_tensor(
                scr[:], wsb[:, k, :], 1.0, ab[:, :F], Alu.mult, Alu.mult,
                accum_out=wa[:, k, 1:2],
            )
        # rhs16 cols: [W@a2 | W | W@a1] so psh comes out [s2 | h]
        rhs16 = sbP.tile([128, nk, F + 2], fp16)
        for k in range(nk):
            nc.vector.tensor_copy(rhs16[:, k, 0:1], wa[:, k, 0:1])
            nc.vector.tensor_copy(rhs16[:, k, 1 : F + 1], wsb[:, k, :])
            nc.vector.tensor_copy(rhs16[:, k, F + 1 : F + 2], wa[:, k, 1:2])

        # ---- s1 local -> DRAM bounce -> free-axis broadcast tile ----------
        s1c16 = sbP.tile([128, nit], fp16)
        for it in range(nit):
            ps1 = pp.tile([128, 1], f32, tag="pro", name=f"ps1_{it}", bufs=2)
            for k in range(nk):
                nc.tensor.matmul(
                    ps1[:], xTl[:, k, it * 128 : (it + 1) * 128],
                    rhs16[:, k, F + 1 : F + 2],
                    start=(k == 0), stop=(k == nk - 1),
                )
            nc.vector.tensor_copy(s1c16[:, it : it + 1], ps1[:])
        # s1 column -> row without a DRAM bounce: PE transpose, then one
        # tiny SBUF->SBUF DMA to flatten the nit partitions into one row
        pst = pp.tile([nit, 128], fp16, tag="pro", name="pst", bufs=2)
        nc.tensor.transpose(pst[:], s1c16[:], ident128[:])
        psrowSB = sbP.tile([nit, 128], fp16)
        nc.vector.tensor_copy(psrowSB[:], pst[:])
        s1row = sbP.tile([1, rl], fp16)
        nc.sync.dma_start(
            s1row[:].rearrange("o (t i) -> o t i", t=nit), psrowSB[:]
        )
        ones1 = sbP.tile([1, 128], fp16)
        nc.vector.memset(ones1[:], 1.0)
        s1b = sbP.tile([128, rl], fp16)
        for cc0 in range(0, rl, 512):
            wch = min(512, rl - cc0)
            psb = pp.tile([128, wch], f32, tag="pro", name=f"psb{cc0}", bufs=2)
            nc.tensor.matmul(psb[:], ones1[:], s1row[:, cc0 : cc0 + wch])
            nc.vector.tensor_copy(s1b[:, cc0 : cc0 + wch], psb[:])

        # ---- hs16 [s2|h|ones]; stationary slice is cols 1: = [h|ones] -----
        hs8 = sbP.tile([128, njt, F + 2], fp16)
        nc.vector.memset(hs8[:, :, F + 1 : F + 2], 1.0)
        s2c = sbP.tile([128, njt], f32)

        w_lo = min(512, rl)
        pso_lo = pp.tile([HW, w_lo], f32, tag="lo", name="pso_lo")
        pso_hi = pp.tile([HW, rl - 512], f32, tag="hi", name="pso_hi") \
            if rl > 512 else None

        # hs emitted in half-chunk units, ~2 quads ahead of first use, so a
        # quad never queues behind more than half a chunk of copies/casts
        hpc = max(1, tpc // 2)    # j-tiles per emission unit
        nun = njt // hpc
        done_units = 0

        for q in range(NQ):
            need = min(nun, ((q + 2) * QT + hpc - 1) // hpc)
            while done_units < need:
                u = done_units
                c = (u * hpc) // tpc
                for tth in range(hpc):
                    t = u * hpc + tth
                    psh = pp.tile([128, F + 1], f32, tag="psh", name=f"psh{t}",
                                  bufs=2)
                    for k in range(nk):
                        nc.tensor.matmul(
                            psh[:],
                            xTf[c][:, k, (t - c * tpc) * 128
                                   : (t - c * tpc + 1) * 128],
                            rhs16[:, k, 0 : F + 1],
                            start=(k == 0), stop=(k == nk - 1),
                        )
                    nc.scalar.copy(hs8[:, t, 0 : F + 1], psh[:])
                sl = slice(u * hpc, (u + 1) * hpc)
                nc.vector.tensor_copy(s2c[:, sl], hs8[:, sl, 0])
                done_units += 1

            w = aq[q]  # in-place: adj tile becomes the masked-P tile
            u = sbU.tile([128, QT, rl], fp16, tag="u", name=f"u{q}")
            for tt in range(QT):
                t = q * QT + tt
                nc.vector.tensor_scalar(
                    u[:, tt, :], s1b[:], s2c[:, t : t + 1], None, Alu.add
                )
            wf = w[:].rearrange("p t i -> p (t i)")
            uf = u[:].rearrange("p t i -> p (t i)")
            if q == NQ - 1 and not SIM_SAFE:
                # last quad: per-tile stages so the drained engines pipeline
                # the final chain instead of serializing [128,4096]-wide ops
                for tt in range(QT):
                    ut = u[:, tt, :]
                    if tt < QT // 2:
                        nc.scalar.activation(
                            ut, ut, Act.Prelu, bias=0.0, scale=1.0, alpha=ALPHA
                        )
                    else:
                        lt = sbU.tile([128, rl], fp16, tag="lt",
                                      name=f"ltz{tt}", bufs=2)
                        nc.vector.tensor_scalar_mul(lt[:], ut, ALPHA)
                        nc.vector.tensor_tensor(ut, ut, lt[:], Alu.max)
                    nc.scalar.activation(ut, ut, Act.Exp, bias=cshift[:],
                                         scale=1.0)
                    nc.vector.tensor_tensor(w[:, tt, :], w[:, tt, :], ut,
                                            Alu.mult)
            else:
                # leaky_relu: front span on ACT (exact Prelu), rest native DVE
                sp_a = 0 if SIM_SAFE else (QT * rl) // 2
                if sp_a:
                    nc.scalar.activation(
                        uf[:, 0:sp_a], uf[:, 0:sp_a], Act.Prelu,
                        bias=0.0, scale=1.0, alpha=ALPHA,
                    )
                ud = uf[:, sp_a : QT * rl]
                lt = sbU.tile([128, QT * rl - sp_a], fp16, tag="lt",
                              name=f"lt{q}", bufs=2)
                nc.vector.tensor_scalar_mul(lt[:], ud, ALPHA)
                nc.vector.tensor_tensor(ud, ud, lt[:], Alu.max)
                nc.scalar.activation(uf, uf, Act.Exp, bias=cshift[:], scale=1.0)
                nc.vector.tensor_tensor(wf, wf, uf, Alu.mult)
            for tt in range(QT):
                t = q * QT + tt
                st, sp = (t == 0), (t == njt - 1)
                nc.tensor.matmul(
                    pso_lo[:], hs8[:, t, 1 : F + 2], w[:, tt, 0:w_lo],
                    start=st, stop=sp,
                )
                if pso_hi is not None:
                    nc.tensor.matmul(
                        pso_hi[:], hs8[:, t, 1 : F + 2], w[:, tt, 512:rl],
                        start=st, stop=sp,
                    )

        # ---- epilogue: transpose psoT back, divide, elu -------------------
        psoSB = sbP.tile([HW, rl], f32)
        nc.vector.tensor_copy(psoSB[:, 0:w_lo], pso_lo[:])
        if pso_hi is not None:
            nc.vector.tensor_copy(psoSB[:, 512:rl], pso_hi[:])
        # per-chunk: transpose + divide; elu batched across all chunks so the
        # tail is 4 wide ops + one DMA instead of 32 tiny ops + 8 DMAs
        stage = sbE.tile([128, nit, F], f32, tag="stage", bufs=1)
        for it in range(nit):
            psT = pp.tile([128, HW], f32, tag="psT", name=f"psT{it}", bufs=2)
            nc.tensor.transpose(
                psT[:], psoSB[:, it * 128 : (it + 1) * 128], ident[:]
            )
            rcp = sbE.tile([128, 1], f32, tag="rcp")
            nc.vector.reciprocal(rcp[:], psT[:, F : F + 1])
            nc.vector.tensor_scalar_mul(stage[:, it, :], psT[:, 0:F], rcp[:])
        sf = stage[:].rearrange("p t f -> p (t f)")
        rt = sbE.tile([128, nit * F], f32, tag="rt", bufs=1)
        nc.vector.tensor_scalar_max(rt[:], sf, 0.0)
        nc.vector.tensor_scalar_min(sf, sf, 0.0)
        nc.scalar.activation(sf, sf, Act.Exp)
        nc.vector.scalar_tensor_tensor(rt[:], sf, -1.0, rt[:], Alu.add, Alu.add)
        nc.sync.dma_start(
            out_d.rearrange("(t p) f -> p t f", p=128),
            rt[:].rearrange("p (t f) -> p t f", t=nit),
        )


def _build(n=N, rl=RL, ncores=NCORES):
    key = (n, rl, ncores)
    if key in _CACHE:
        return _CACHE[key]
    nc = bacc.Bacc(
        "TRN2", target_bir_lowering=False, debug=False, num_devices=ncores
    )
    featT = nc.dram_tensor("featT", [D, n], f32, kind="ExternalInput").ap()
    featTl = nc.dram_tensor("featTl", [D, rl], f32, kind="ExternalInput").ap()
    adjT = nc.dram_tensor("adjT", [n, rl], i32, kind="ExternalInput").ap()
    W = nc.dram_tensor("W", [D, F], f32, kind="ExternalInput").ap()
    a = nc.dram_tensor("a", [2 * F, 1], f32, kind="ExternalInput").ap()
    out = nc.dram_tensor("out", [rl, F], f32, kind="ExternalOutput").ap()
    with tile.TileContext(nc) as tc:
        _kernel_body(tc, out, featT, featTl, adjT, W, a, n=n, rl=rl)
    nc.compile()
    _CACHE[key] = nc
    return nc


def kernel(features, adj, W, a):
    global LAST_RESULTS
    features = np.ascontiguousarray(features, dtype=np.float32)
    adj = np.ascontiguousarray(adj, dtype=np.int32)
    W = np.ascontiguousarray(W, dtype=np.float32)
    a = np.ascontiguousarray(a, dtype=np.float32)

    n = adj.shape[0]
    rl = n // NCORES
    nc = _build(n=n, rl=rl, ncores=NCORES)
    featT = np.ascontiguousarray(features.T)
    in_maps = [
        {
            "featT": featT,
            "featTl": np.ascontiguousarray(features[c * rl : (c + 1) * rl].T),
            "adjT": np.ascontiguousarray(adj[c * rl : (c + 1) * rl].T),
            "W": W,
            "a": a,
        }
        for c in range(NCORES)
    ]
    res = bass_utils.run_bass_kernel_spmd(nc, in_maps, core_ids=list(range(NCORES)))
    LAST_RESULTS = res
    return np.concatenate([res.results[c]["out"] for c in range(NCORES)], axis=0)
